# revision 4
# baseline (speedup 1.0000x reference)
"""Single-head causal self-attention on 8 Trainium2 NeuronCores.

Problem: x[8, 2048, 1024], Wq/Wk/Wv[1024, 64] ->
  out[b] = softmax(causal((x[b]@Wq) @ (x[b]@Wk)^T / 8)) @ (x[b]@Wv)

Sharding: data-parallel over batch B=8, one batch element per core; weights
replicated. x is transposed host-side per core and Wq|Wk are concatenated so
every on-device matmul contracts over the SBUF partition dim with dense DMAs.

Per-core scheme ("transposed scores"):
  - [q^T;k^T] = Wqk^T @ x^T   (PE, fused, evacuated into two base-0 tiles via
    partition-shifted ACT copies)
  - v^T = Wv^T @ x^T, then PE-transpose -> V[2048, 64] (+ ones column)
  - S^T[j-tile, q-chunk] = (k^T tile)^T @ q^T, causal blocks only
  - P^T = exp(S^T / 8)  (ACT, PSUM->SBUF; no max-subtraction: scores ~N(0,1))
  - diagonal blocks: multiply boundary 128-col sub-block by a 0/1 triangle
    mask; columns left of it are skipped entirely (matmuls operate on slices)
  - out^T[qc] = sum_j V_aug[j]^T @ P^T ; ones column makes row 64 the softmax
    denominator for free
  - PE-transpose out^T -> [q, 65]; multiply rows by reciprocal of col 64 (DVE)
"""

import numpy as np

import concourse.bass as bass
import concourse.mybir as mybir
import concourse.tile as tile
from concourse import bacc
from concourse.bass_utils import run_bass_kernel_spmd
from concourse.masks import make_identity, make_upper_triangular

N_CORES = 8
B, T, C, D = 8, 2048, 1024, 64
CT = C // 128          # 8 contraction tiles
NT = T // 128          # 16 row tiles
QCHUNK = 512
NQC = T // QCHUNK      # 4 q-chunks
JPER = QCHUNK // 128   # 4 j-tiles per q-chunk
SCALE = float(1.0 / np.sqrt(D))

FP = mybir.dt.float32
MM_DT = mybir.dt.float32r  # matmul ingest dtype; FP = exact but 4x slower


def build_nc():
    nc = bacc.Bacc("TRN2", target_bir_lowering=False)
    xT_h = nc.dram_tensor("xT", [C, T], MM_DT, kind="ExternalInput")
    wqk_h = nc.dram_tensor("wqk", [C, 128], MM_DT, kind="ExternalInput")
    wv_h = nc.dram_tensor("wv", [C, D], MM_DT, kind="ExternalInput")
    y_h = nc.dram_tensor("y", [T, D], FP, kind="ExternalOutput")

    with tile.TileContext(nc) as tc:
        with (
            tc.tile_pool(name="const", bufs=1) as const,
            tc.tile_pool(name="pt", bufs=4) as ptp,
            tc.tile_pool(name="otp", bufs=2) as otp,
            tc.tile_pool(name="ps_s", bufs=2, space="PSUM") as ps_s,
            tc.tile_pool(name="ps_p", bufs=1, space="PSUM") as ps_p,
            tc.tile_pool(name="ps_o", bufs=2, space="PSUM") as ps_o,
            tc.tile_pool(name="ps_t", bufs=2, space="PSUM") as ps_t,
        ):
            # ---- constants ----
            ident = const.tile([128, 128], FP, tag="ident")
            make_identity(nc, ident)
            tri = const.tile([128, 128], FP, tag="tri")  # tri[p,f]=1.0 iff f>=p
            make_upper_triangular(nc, tri, val=1.0, diag=True)

            wqk_sb = const.tile([128, CT, 128], MM_DT, tag="wqk")
            nc.sync.dma_start(
                out=wqk_sb, in_=wqk_h[:, :].rearrange("(ct p) m -> p ct m", p=128)
            )
            wv_sb = const.tile([128, CT, D], MM_DT, tag="wv")
            nc.sync.dma_start(
                out=wv_sb, in_=wv_h[:, :].rearrange("(ct p) m -> p ct m", p=128)
            )

            xT_sb = const.tile([128, CT, T], MM_DT, tag="xT")
            qT = const.tile([64, T], MM_DT, tag="qT")
            kT = const.tile([64, T], MM_DT, tag="kT")
            vT = const.tile([64, T], FP, tag="vT")
            V = const.tile([128, NT, D + 1], MM_DT, tag="V")  # col D = ones
            ones_col = const.tile([128, NT], FP, tag="ones")
            nc.gpsimd.memset(ones_col, 1.0)
            nc.scalar.copy(V[:, :, D], ones_col)
            out_sb = const.tile([128, NT, D], FP, tag="out")

            xT_in = xT_h[:, :].rearrange("(ct p) t -> p ct t", p=128)

            # ---- DMA + projections, pipelined per t-chunk ----
            for tcu in range(NQC):
                sl = slice(tcu * QCHUNK, (tcu + 1) * QCHUNK)
                nc.sync.dma_start(out=xT_sb[:, :, sl], in_=xT_in[:, :, sl])

                p_qk = ps_p.tile([128, QCHUNK], FP, tag="qk")
                for ct in range(CT):
                    nc.tensor.matmul(
                        p_qk,
                        wqk_sb[:, ct, :],
                        xT_sb[:, ct, sl],
                        start=(ct == 0),
                        stop=(ct == CT - 1),
                    )
                nc.scalar.copy(qT[:, sl], p_qk[0:64, :])
                nc.scalar.copy(kT[:, sl], p_qk[64:128, :])  # partition shift

                p_v = ps_p.tile([64, QCHUNK], FP, tag="v")
                for ct in range(CT):
                    nc.tensor.matmul(
                        p_v,
                        wv_sb[:, ct, :],
                        xT_sb[:, ct, sl],
                        start=(ct == 0),
                        stop=(ct == CT - 1),
                    )
                nc.scalar.copy(vT[:, sl], p_v)

                # V natural layout for the j-tiles of this chunk
                for i in range(JPER):
                    jt = tcu * JPER + i
                    p_vt = ps_t.tile([128, D + 1], FP, tag="t")
                    nc.tensor.transpose(
                        p_vt[:, 0:D],
                        vT[:, jt * 128 : (jt + 1) * 128],
                        ident[0:64, 0:64],
                    )
                    nc.scalar.copy(V[:, jt, 0:D], p_vt[:, 0:D])

                # ---- attention for q-chunk qc = tcu (needs k/v chunks <= tcu) ----
                qc = tcu
                p_out = ps_o.tile([D + 1, QCHUNK], FP, tag="o")
                n_jt = qc * JPER + JPER
                blocks = []
                for jt in range(n_jt):
                    i = jt - qc * JPER  # >=0 on diagonal j-tiles
                    lo = max(i, 0) * 128  # first valid column of this block
                    blocks.append((jt, lo))

                def s_block(jt, lo):
                    p_s = ps_s.tile([128, QCHUNK], FP, tag="s")
                    nc.tensor.matmul(
                        p_s[:, lo:QCHUNK],
                        kT[:, jt * 128 : (jt + 1) * 128],
                        qT[:, qc * QCHUNK + lo : (qc + 1) * QCHUNK],
                        start=True,
                        stop=True,
                    )
                    pt = ptp.tile([128, QCHUNK], MM_DT, tag="pt")
                    nc.scalar.activation(
                        pt[:, lo:QCHUNK],
                        p_s[:, lo:QCHUNK],
                        mybir.ActivationFunctionType.Exp,
                        scale=SCALE,
                    )
                    if jt - qc * JPER >= 0:
                        nc.vector.tensor_mul(
                            pt[:, lo : lo + 128], pt[:, lo : lo + 128], tri
                        )
                    return pt

                # software pipeline: keep one S block in flight ahead of AV
                pts = {}
                pts[0] = s_block(*blocks[0])
                for idx, (jt, lo) in enumerate(blocks):
                    if idx + 1 < len(blocks):
                        pts[idx + 1] = s_block(*blocks[idx + 1])
                    pt = pts.pop(idx)
                    nc.tensor.matmul(
                        p_out[:, lo:QCHUNK],
                        V[:, jt, :],
                        pt[:, lo:QCHUNK],
                        start=(jt == 0),
                        stop=(jt == n_jt - 1),
                    )

                # ---- normalize + transpose back to [q, d] ----
                oT = otp.tile([D + 1, QCHUNK], FP, tag="ot")
                nc.scalar.copy(oT, p_out)
                for i in range(JPER):
                    qt = qc * JPER + i
                    p_tr = ps_t.tile([128, D + 1], FP, tag="t")
                    nc.tensor.transpose(
                        p_tr,
                        oT[:, i * 128 : (i + 1) * 128],
                        ident[0 : D + 1, 0 : D + 1],
                    )
                    s_sb = otp.tile([128, 2], FP, tag="s_sb")
                    nc.vector.tensor_copy(s_sb[:, 0:1], p_tr[:, D : D + 1])
                    nc.vector.reciprocal(s_sb[:, 1:2], s_sb[:, 0:1])
                    nc.vector.tensor_scalar_mul(
                        out_sb[:, qt, :], p_tr[:, 0:D], s_sb[:, 1:2]
                    )

            nc.sync.dma_start(
                out=y_h[:, :].rearrange("(qt p) d -> p qt d", p=128), in_=out_sb
            )

    nc.finalize()
    return nc


_NC_CACHE = None
LAST_RESULTS = None


def kernel(x, Wq, Wk, Wv, trace=False, **run_kwargs):
    global _NC_CACHE, LAST_RESULTS
    x = np.ascontiguousarray(np.asarray(x, dtype=np.float32))
    wqk = np.ascontiguousarray(
        np.concatenate(
            [np.asarray(Wq, np.float32), np.asarray(Wk, np.float32)], axis=1
        )
    )
    wv = np.ascontiguousarray(np.asarray(Wv, dtype=np.float32))

    if _NC_CACHE is None:
        _NC_CACHE = build_nc()
    nc = _NC_CACHE

    in_maps = [
        {"xT": np.ascontiguousarray(x[b].T), "wqk": wqk, "wv": wv}
        for b in range(N_CORES)
    ]
    res = run_bass_kernel_spmd(
        nc, in_maps, core_ids=list(range(N_CORES)), trace=trace, **run_kwargs
    )
    LAST_RESULTS = res
    return np.stack([res.results[b]["y"] for b in range(N_CORES)], axis=0)


if __name__ == "__main__":
    rng = np.random.default_rng(0)
    x = rng.standard_normal((B, T, C), dtype=np.float32)
    s = 1.0 / np.sqrt(C)
    Wq = rng.standard_normal((C, D), dtype=np.float32) * s
    Wk = rng.standard_normal((C, D), dtype=np.float32) * s
    Wv = rng.standard_normal((C, D), dtype=np.float32) * s
    out = kernel(x, Wq, Wk, Wv)
    print("out", out.shape, out.dtype, float(np.abs(out).max()))


# revision 5
# speedup vs baseline: 1.0051x; 1.0051x over previous
"""Single-head causal self-attention on 8 Trainium2 NeuronCores.

Problem: x[8, 2048, 1024], Wq/Wk/Wv[1024, 64] ->
  out[b] = softmax(causal((x[b]@Wq) @ (x[b]@Wk)^T / 8)) @ (x[b]@Wv)

Sharding: data-parallel over batch B=8, one batch element per core; weights
replicated. x is transposed host-side per core and Wq|Wk are concatenated so
every on-device matmul contracts over the SBUF partition dim with dense DMAs.

Per-core scheme ("transposed scores"):
  - [q^T;k^T] = Wqk^T @ x^T   (PE, fused, evacuated into two base-0 tiles via
    partition-shifted ACT copies)
  - v^T = Wv^T @ x^T, then PE-transpose -> V[2048, 64] (+ ones column)
  - S^T[j-tile, q-chunk] = (k^T tile)^T @ q^T, causal blocks only
  - P^T = exp(S^T / 8)  (ACT, PSUM->SBUF; no max-subtraction: scores ~N(0,1))
  - diagonal blocks: multiply boundary 128-col sub-block by a 0/1 triangle
    mask; columns left of it are skipped entirely (matmuls operate on slices)
  - out^T[qc] = sum_j V_aug[j]^T @ P^T ; ones column makes row 64 the softmax
    denominator for free
  - PE-transpose out^T -> [q, 65]; multiply rows by reciprocal of col 64 (DVE)
"""

import numpy as np

import concourse.bass as bass
import concourse.mybir as mybir
import concourse.tile as tile
from concourse import bacc
from concourse.bass_utils import run_bass_kernel_spmd
from concourse.masks import make_identity, make_upper_triangular

N_CORES = 8
B, T, C, D = 8, 2048, 1024, 64
CT = C // 128          # 8 contraction tiles
NT = T // 128          # 16 row tiles
QCHUNK = 512
NQC = T // QCHUNK      # 4 q-chunks
JPER = QCHUNK // 128   # 4 j-tiles per q-chunk
SCALE = float(1.0 / np.sqrt(D))

FP = mybir.dt.float32
MM_DT = mybir.dt.float32r  # matmul ingest dtype; FP = exact but 4x slower


def build_nc():
    nc = bacc.Bacc("TRN2", target_bir_lowering=False)
    xT_h = nc.dram_tensor("xT", [C, T], MM_DT, kind="ExternalInput")
    wqk_h = nc.dram_tensor("wqk", [C, 128], MM_DT, kind="ExternalInput")
    wv_h = nc.dram_tensor("wv", [C, D], MM_DT, kind="ExternalInput")
    y_h = nc.dram_tensor("y", [T, D], FP, kind="ExternalOutput")

    with tile.TileContext(nc) as tc:
        with (
            tc.tile_pool(name="const", bufs=1) as const,
            tc.tile_pool(name="pt", bufs=4) as ptp,
            tc.tile_pool(name="otp", bufs=2) as otp,
            tc.tile_pool(name="ps_s", bufs=3, space="PSUM") as ps_s,
            tc.tile_pool(name="ps_p", bufs=1, space="PSUM") as ps_p,
            tc.tile_pool(name="ps_o", bufs=2, space="PSUM") as ps_o,
            tc.tile_pool(name="ps_t", bufs=1, space="PSUM") as ps_t,
        ):
            # ---- constants ----
            ident = const.tile([128, 128], FP, tag="ident")
            make_identity(nc, ident)
            tri = const.tile([128, 128], FP, tag="tri")  # tri[p,f]=1.0 iff f>=p
            make_upper_triangular(nc, tri, val=1.0, diag=True)

            wqk_sb = const.tile([128, CT, 128], MM_DT, tag="wqk")
            nc.sync.dma_start(
                out=wqk_sb, in_=wqk_h[:, :].rearrange("(ct p) m -> p ct m", p=128)
            )
            wv_sb = const.tile([128, CT, D], MM_DT, tag="wv")
            nc.sync.dma_start(
                out=wv_sb, in_=wv_h[:, :].rearrange("(ct p) m -> p ct m", p=128)
            )

            xT_sb = const.tile([128, CT, T], MM_DT, tag="xT")
            qT = const.tile([64, T], MM_DT, tag="qT")
            kT = const.tile([64, T], MM_DT, tag="kT")
            vT = const.tile([64, T], FP, tag="vT")
            V = const.tile([128, NT, D + 1], MM_DT, tag="V")  # col D = ones
            ones_col = const.tile([128, NT], FP, tag="ones")
            nc.gpsimd.memset(ones_col, 1.0)
            nc.scalar.copy(V[:, :, D], ones_col)
            out_sb = const.tile([128, NT, D], FP, tag="out")

            xT_in = xT_h[:, :].rearrange("(ct p) t -> p ct t", p=128)

            # ---- DMA + projections, pipelined per t-chunk ----
            for tcu in range(NQC):
                sl = slice(tcu * QCHUNK, (tcu + 1) * QCHUNK)
                nc.sync.dma_start(out=xT_sb[:, :, sl], in_=xT_in[:, :, sl])

                p_qk = ps_p.tile([128, QCHUNK], FP, tag="qk")
                for ct in range(CT):
                    nc.tensor.matmul(
                        p_qk,
                        wqk_sb[:, ct, :],
                        xT_sb[:, ct, sl],
                        start=(ct == 0),
                        stop=(ct == CT - 1),
                    )
                nc.scalar.copy(qT[:, sl], p_qk[0:64, :])
                nc.scalar.copy(kT[:, sl], p_qk[64:128, :])  # partition shift

                p_v = ps_p.tile([64, QCHUNK], FP, tag="v")
                for ct in range(CT):
                    nc.tensor.matmul(
                        p_v,
                        wv_sb[:, ct, :],
                        xT_sb[:, ct, sl],
                        start=(ct == 0),
                        stop=(ct == CT - 1),
                    )
                nc.scalar.copy(vT[:, sl], p_v)

                # V natural layout for the j-tiles of this chunk
                for i in range(JPER):
                    jt = tcu * JPER + i
                    p_vt = ps_t.tile([128, D + 1], FP, tag="t")
                    nc.tensor.transpose(
                        p_vt[:, 0:D],
                        vT[:, jt * 128 : (jt + 1) * 128],
                        ident[0:64, 0:64],
                    )
                    nc.scalar.copy(V[:, jt, 0:D], p_vt[:, 0:D])

                # ---- attention for q-chunk qc = tcu (needs k/v chunks <= tcu) ----
                qc = tcu
                p_out = ps_o.tile([D + 1, QCHUNK], FP, tag="o")
                n_jt = qc * JPER + JPER
                blocks = []
                for jt in range(n_jt):
                    i = jt - qc * JPER  # >=0 on diagonal j-tiles
                    lo = max(i, 0) * 128  # first valid column of this block
                    blocks.append((jt, lo))

                def s_block(jt, lo):
                    p_s = ps_s.tile([128, QCHUNK], FP, tag="s")
                    nc.tensor.matmul(
                        p_s[:, lo:QCHUNK],
                        kT[:, jt * 128 : (jt + 1) * 128],
                        qT[:, qc * QCHUNK + lo : (qc + 1) * QCHUNK],
                        start=True,
                        stop=True,
                    )
                    pt = ptp.tile([128, QCHUNK], MM_DT, tag="pt")
                    nc.scalar.activation(
                        pt[:, lo:QCHUNK],
                        p_s[:, lo:QCHUNK],
                        mybir.ActivationFunctionType.Exp,
                        scale=SCALE,
                    )
                    if jt - qc * JPER >= 0:
                        nc.vector.tensor_mul(
                            pt[:, lo : lo + 128], pt[:, lo : lo + 128], tri
                        )
                    return pt

                # software pipeline: keep one S block in flight ahead of AV
                AHEAD = 2
                pts = {}
                for k in range(min(AHEAD, len(blocks))):
                    pts[k] = s_block(*blocks[k])
                for idx, (jt, lo) in enumerate(blocks):
                    if idx + AHEAD < len(blocks):
                        pts[idx + AHEAD] = s_block(*blocks[idx + AHEAD])
                    pt = pts.pop(idx)
                    nc.tensor.matmul(
                        p_out[:, lo:QCHUNK],
                        V[:, jt, :],
                        pt[:, lo:QCHUNK],
                        start=(jt == 0),
                        stop=(jt == n_jt - 1),
                    )

                # ---- normalize + transpose back to [q, d] ----
                oT = otp.tile([D + 1, QCHUNK], FP, tag="ot")
                nc.scalar.copy(oT, p_out)
                for i in range(JPER):
                    qt = qc * JPER + i
                    p_tr = ps_t.tile([128, D + 1], FP, tag="t")
                    nc.tensor.transpose(
                        p_tr,
                        oT[:, i * 128 : (i + 1) * 128],
                        ident[0 : D + 1, 0 : D + 1],
                    )
                    s_sb = otp.tile([128, 2], FP, tag="s_sb")
                    nc.vector.tensor_copy(s_sb[:, 0:1], p_tr[:, D : D + 1])
                    nc.vector.reciprocal(s_sb[:, 1:2], s_sb[:, 0:1])
                    nc.vector.tensor_scalar_mul(
                        out_sb[:, qt, :], p_tr[:, 0:D], s_sb[:, 1:2]
                    )

            nc.sync.dma_start(
                out=y_h[:, :].rearrange("(qt p) d -> p qt d", p=128), in_=out_sb
            )

    nc.finalize()
    return nc


_NC_CACHE = None
LAST_RESULTS = None


def kernel(x, Wq, Wk, Wv, trace=False, **run_kwargs):
    global _NC_CACHE, LAST_RESULTS
    x = np.ascontiguousarray(np.asarray(x, dtype=np.float32))
    wqk = np.ascontiguousarray(
        np.concatenate(
            [np.asarray(Wq, np.float32), np.asarray(Wk, np.float32)], axis=1
        )
    )
    wv = np.ascontiguousarray(np.asarray(Wv, dtype=np.float32))

    if _NC_CACHE is None:
        _NC_CACHE = build_nc()
    nc = _NC_CACHE

    in_maps = [
        {"xT": np.ascontiguousarray(x[b].T), "wqk": wqk, "wv": wv}
        for b in range(N_CORES)
    ]
    res = run_bass_kernel_spmd(
        nc, in_maps, core_ids=list(range(N_CORES)), trace=trace, **run_kwargs
    )
    LAST_RESULTS = res
    return np.stack([res.results[b]["y"] for b in range(N_CORES)], axis=0)


if __name__ == "__main__":
    rng = np.random.default_rng(0)
    x = rng.standard_normal((B, T, C), dtype=np.float32)
    s = 1.0 / np.sqrt(C)
    Wq = rng.standard_normal((C, D), dtype=np.float32) * s
    Wk = rng.standard_normal((C, D), dtype=np.float32) * s
    Wv = rng.standard_normal((C, D), dtype=np.float32) * s
    out = kernel(x, Wq, Wk, Wv)
    print("out", out.shape, out.dtype, float(np.abs(out).max()))
